# revision 28
# baseline (speedup 1.0000x reference)
"""Causal single-head attention (B=4, T=4096, C=2048, H=128) on 8 TRN2 cores.

Sharding: data-parallel over batch (2 cores per batch element). Within a
batch, core half h owns query tiles qt with qt mod 4 in {2h, 2h+1} — both
cores get an identical multiset of causal key-block counts, so one SPMD
program is balanced. Each core also projects k/v only for its own 2048
columns; the halves are exchanged per 512-column group with a pair-wise
AllGather, halving both the x DMA traffic and the k/v projection FLOPs.

Per-core device program (fp16 operands, f32 PSUM accumulation), pipelined
per column group g: project k^T/v^T/q^T of my 256 columns from slab g,
AllGather (k^T|v^T) with the pair partner, then after every odd group run
one attention q-group (4 query tiles, 512 q columns) in the transposed
S^T layout:
  S^T chunk [s=128, q=512] (PE) -> exp (ACT) -> x 0/1 causal mask (DVE) ->
  row-sums via ones-matmul (PE, replicated rows) + out^T AV accumulation
  (PE) -> out^T * (1/sums) (DVE) -> PE-transpose -> +bv (DVE) -> DMA out.
"""

import numpy as np

import concourse.bacc as bacc
import concourse.mybir as mybir
import concourse.tile as tile
from concourse.bass_utils import run_bass_kernel_spmd

B, T, C, H = 4, 4096, 2048, 128
P = 128          # partitions / head dim / q tile
KB = 512         # free-dim tile (one f32 PSUM bank)
HB = 256         # per-core half of a column group
NQT = 16         # query tiles per core
TQ = NQT * P     # query rows per core
NCC = C // P     # contraction chunks (16)
NG = T // KB     # 512-wide column groups (8)
NM = 4           # attention q-groups per core (4 tiles each)

F16 = np.float16
_NC_CACHE = {}
REPLICA_GROUPS = [[0, 1], [2, 3], [4, 5], [6, 7]]


def _qtiles_for(half):
    # global query-tile ids, j-th tile of this core; kb counts [1,1,2,2,...,8,8]
    return [4 * (j // 2) + 2 * half + (j % 2) for j in range(NQT)]


def build_nc():
    dt = mybir.dt
    nc = bacc.Bacc("TRN2", target_bir_lowering=False, debug=False, num_devices=8)

    xP = nc.dram_tensor("xP", [NG, P, NCC, HB], dt.float16, kind="ExternalInput").ap()
    wk = nc.dram_tensor("wk", [P, NCC, H], dt.float16, kind="ExternalInput").ap()
    wq = nc.dram_tensor("wq", [P, NCC, H], dt.float16, kind="ExternalInput").ap()
    wv = nc.dram_tensor("wv", [P, NCC, H], dt.float16, kind="ExternalInput").ap()
    bk = nc.dram_tensor("bk", [P, 1], dt.float32, kind="ExternalInput").ap()
    bq = nc.dram_tensor("bq", [P, 1], dt.float32, kind="ExternalInput").ap()
    bvb = nc.dram_tensor("bvb", [P, H], dt.float16, kind="ExternalInput").ap()
    consts = nc.dram_tensor(
        "consts", [P, 2, P], dt.float16, kind="ExternalInput"
    ).ap()
    masks = nc.dram_tensor(
        "masks", [NM * 4, P, 2 * KB], dt.float16, kind="ExternalInput"
    ).ap()
    out = nc.dram_tensor("out", [TQ, H], dt.float32, kind="ExternalOutput").ap()

    Exp = mybir.ActivationFunctionType.Exp
    Ident = mybir.ActivationFunctionType.Identity

    with tile.TileContext(nc) as tc:
        with (
            tc.tile_pool(name="wpool", bufs=1) as wpool,
            tc.tile_pool(name="persist", bufs=1) as persist,
            tc.tile_pool(name="xpool", bufs=6) as xpool,
            tc.tile_pool(name="vtpool", bufs=8) as vtpool,
            tc.tile_pool(name="kvpool", bufs=2) as kvpool,
            tc.tile_pool(name="dram", bufs=8, space="DRAM") as dram,
            tc.tile_pool(name="bank512", bufs=2, space="PSUM") as spool,
            tc.tile_pool(name="sumpool", bufs=1, space="PSUM") as sumpool,
            tc.tile_pool(name="outTpool", bufs=1, space="PSUM") as otpool,
            tc.tile_pool(name="bank128", bufs=2, space="PSUM") as tpool,
            tc.tile_pool(name="weipool", bufs=5) as weipool,
            tc.tile_pool(name="mpool", bufs=16) as mpool,
            tc.tile_pool(name="stat", bufs=2) as stat,
            tc.tile_pool(name="osbpool", bufs=2) as osbpool,
            tc.tile_pool(name="opool", bufs=4) as opool,
            tc.tile_pool(name="cpool", bufs=1) as cpool,
        ):
            # ---- constants (all host-pre-tiled: contiguous per partition) ----
            wk_t = wpool.tile([P, NCC, H], dt.float16, tag="wk")
            wq_t = wpool.tile([P, NCC, H], dt.float16, tag="wq")
            wv_t = wpool.tile([P, NCC, H], dt.float16, tag="wv")
            nc.sync.dma_start(wk_t[:], wk)
            nc.sync.dma_start(wq_t[:], wq)
            nc.sync.dma_start(wv_t[:], wv)
            bk_t = cpool.tile([P, 1], dt.float32, tag="bk")
            bq_t = cpool.tile([P, 1], dt.float32, tag="bq")
            bvb_t = cpool.tile([P, H], dt.float16, tag="bvb")
            nc.sync.dma_start(bk_t[:], bk)
            nc.sync.dma_start(bq_t[:], bq)
            nc.sync.dma_start(bvb_t[:], bvb)
            idon = cpool.tile([P, 2, P], dt.float16, tag="idon")
            nc.sync.dma_start(idon[:], consts)

            kT = persist.tile([P, T], dt.float16, tag="kT")
            qT = persist.tile([P, TQ], dt.float16, tag="qT")
            vS = persist.tile([P, T // P, H], dt.float16, tag="vS")

            def project(g):
                xs = xpool.tile([P, NCC, HB], dt.float16, tag="xs")
                nc.sync.dma_start(xs[:], xP[g])
                # k^T|v^T of my half -> packed SBUF tile for the exchange
                kv = kvpool.tile([P, 2, HB], dt.float16, tag="kv")
                pk = spool.tile([P, HB], dt.float32, tag="bank512")
                for cc in range(NCC):
                    nc.tensor.matmul(
                        pk[:], lhsT=wk_t[:, cc, :], rhs=xs[:, cc, :],
                        start=(cc == 0), stop=(cc == NCC - 1),
                    )
                nc.scalar.activation(kv[:, 0, :], pk[:], Ident, bias=bk_t[:])
                pv = spool.tile([P, HB], dt.float32, tag="bank512")
                for cc in range(NCC):
                    nc.tensor.matmul(
                        pv[:], lhsT=wv_t[:, cc, :], rhs=xs[:, cc, :],
                        start=(cc == 0), stop=(cc == NCC - 1),
                    )
                vt = vtpool.tile([P, HB], dt.float16, tag="vt")
                nc.scalar.copy(vt[:], pv[:])
                for s4 in range(2):
                    tp = tpool.tile([P, P], dt.float16, tag="bank128")
                    nc.tensor.transpose(
                        tp[:], vt[:, P * s4 : P * (s4 + 1)], idon[:, 0, :]
                    )
                    nc.vector.tensor_copy(kv[:, 1, P * s4 : P * (s4 + 1)], tp[:])
                # q^T for my two tiles
                pq = spool.tile([P, HB], dt.float32, tag="bank512")
                for cc in range(NCC):
                    nc.tensor.matmul(
                        pq[:], lhsT=wq_t[:, cc, :], rhs=xs[:, cc, :],
                        start=(cc == 0), stop=(cc == NCC - 1),
                    )
                nc.scalar.activation(
                    qT[:, HB * g : HB * (g + 1)], pq[:], Ident, bias=bq_t[:],
                )
                # pair-wise exchange of (k^T | v^T) halves
                cin = dram.tile([P, 2, HB], dt.float16, tag="cin")
                cout = dram.tile([2, P, 2, HB], dt.float16, tag="cout")
                nc.gpsimd.dma_start(cin[:], kv[:])
                nc.gpsimd.collective_compute(
                    "AllGather",
                    mybir.AluOpType.bypass,
                    replica_groups=REPLICA_GROUPS,
                    ins=[cin.opt()],
                    outs=[cout.opt()],
                )
                return cout

            def attention(m, mts):
                nch = (2 * m + 2) * 4       # 128-wide key chunks for this group
                npr = nch // 2
                sums = sumpool.tile([P, KB], dt.float32, tag="sums")
                otp = otpool.tile([P, KB], dt.float32, tag="outT")
                qg = qT[:, KB * m : KB * (m + 1)]
                wei_tiles = []

                def ones_av(p):
                    w = wei_tiles[p]
                    for h2 in range(2):
                        c = 2 * p + h2
                        nc.tensor.matmul(
                            sums[:], lhsT=idon[:, 1, :], rhs=w[:, h2, :],
                            start=(c == 0), stop=(c == nch - 1),
                        )
                        nc.tensor.matmul(
                            otp[:], lhsT=vS[:, c, :], rhs=w[:, h2, :],
                            start=(c == 0), stop=(c == nch - 1),
                        )

                for p in range(npr):
                    st = spool.tile([P, 2, KB], dt.float32, tag="bank512")
                    for h2 in range(2):
                        nc.tensor.matmul(
                            st[:, h2, :],
                            lhsT=kT[:, P * (2 * p + h2) : P * (2 * p + h2 + 1)],
                            rhs=qg, start=True, stop=True,
                        )
                    wei = weipool.tile([P, 2, KB], dt.float16, tag="wei")
                    nc.scalar.activation(wei[:], st[:], Exp)
                    if p >= npr - 4:
                        mt = mts[4 * m + (p - (npr - 4))]
                        nc.vector.tensor_mul(wei[:], wei[:], mt[:])
                    wei_tiles.append(wei)
                    if p > 0:
                        ones_av(p - 1)
                ones_av(npr - 1)
                rec = stat.tile([P, KB], dt.float32, tag="rec")
                nc.vector.reciprocal(rec[:], sums[:])
                osb = osbpool.tile([P, KB], dt.float16, tag="osb")
                nc.vector.tensor_mul(osb[:], otp[:], rec[:])
                for r in range(4):
                    tp = tpool.tile([P, P], dt.float16, tag="bank128")
                    nc.tensor.transpose(
                        tp[:], osb[:, P * r : P * (r + 1)], idon[:, 0, :]
                    )
                    ot = opool.tile([P, H], dt.float32, tag="ot")
                    nc.vector.tensor_add(ot[:], tp[:], bvb_t[:])
                    j = 4 * m + r
                    nc.sync.dma_start(out[P * j : P * (j + 1), :], ot[:])

            couts = [project(g) for g in range(NG)]
            mts = []
            for i in range(NM * 4):
                mt = mpool.tile([P, 2, KB], dt.float16, tag="mask")
                nc.sync.dma_start(mt[:], masks[i])
                mts.append(mt)
            for g, cout in enumerate(couts):
                for r in range(2):
                    nc.sync.dma_start(
                        kT[:, KB * g + HB * r : KB * g + HB * (r + 1)],
                        cout[r, :, 0, :],
                    )
                    for s4 in range(2):
                        nc.sync.dma_start(
                            vS[:, 4 * g + 2 * r + s4, :],
                            cout[r, :, 1, P * s4 : P * (s4 + 1)],
                        )
            for m in range(NM - 1, -1, -1):
                attention(m, mts)

    nc.compile()
    return nc


def _host_prep(x, Wk, bk, Wq, bq, Wv, bv):
    scale = float(C) ** -0.5

    def tile_w(w):
        return np.ascontiguousarray(
            w.reshape(NCC, P, H).transpose(1, 0, 2)
        )

    wk16 = tile_w(np.asarray(Wk, np.float32).astype(F16))
    wq16 = tile_w((np.asarray(Wq, np.float32) * scale).astype(F16))
    wv16 = tile_w(np.asarray(Wv, np.float32).astype(F16))
    bk_c = np.asarray(bk, np.float32).reshape(P, 1)
    bq_c = (np.asarray(bq, np.float32) * scale).reshape(P, 1)
    bvb = np.broadcast_to(np.asarray(bv, np.float32), (P, H)).astype(F16)
    consts = np.ascontiguousarray(
        np.stack([np.eye(P, dtype=F16), np.ones((P, P), F16)]).transpose(1, 0, 2)
    )

    # masks per half: key order is natural global t; 0/1 multiplicative
    masks_by_half = []
    for half in (0, 1):
        qts = _qtiles_for(half)
        m_arr = np.zeros((NM * 4, P, 2, KB), F16)
        for m in range(NM):
            nch = (2 * m + 2) * 4
            qrow = np.empty(KB, np.int64)
            for r in range(4):
                j = 4 * m + r
                qrow[128 * r : 128 * (r + 1)] = qts[j] * P + np.arange(P)
            for k in range(8):
                c = (nch - 8) + k
                keys = 128 * c + np.arange(P)
                m_arr[4 * m + k // 2, :, k % 2, :] = (
                    keys[:, None] <= qrow[None, :]
                ).astype(F16)
        m_arr = m_arr.reshape(NM * 4, P, 2 * KB)
        masks_by_half.append(m_arr)

    in_maps = []
    for core in range(8):
        b_idx, half = core // 2, core % 2
        xTb = np.ascontiguousarray(np.asarray(x[b_idx], np.float32).T).astype(F16)
        xPc = np.empty((NG, P, NCC, HB), F16)
        for g in range(NG):
            grp = xTb[:, KB * g + HB * half : KB * g + HB * (half + 1)]
            xPc[g] = grp.reshape(NCC, P, HB).transpose(1, 0, 2)
        in_maps.append({
            "xP": xPc,
            "wk": wk16, "wq": wq16, "wv": wv16,
            "bk": bk_c, "bq": bq_c, "bvb": bvb,
            "consts": consts, "masks": masks_by_half[half],
        })
    return in_maps


def kernel(x, Wk, bk, Wq, bq, Wv, bv):
    if "nc" not in _NC_CACHE:
        _NC_CACHE["nc"] = build_nc()
    nc = _NC_CACHE["nc"]
    in_maps = _host_prep(x, Wk, bk, Wq, bq, Wv, bv)
    res = run_bass_kernel_spmd(nc, in_maps, list(range(8))).results
    out = np.empty((B, T, H), np.float32)
    for core in range(8):
        b_idx, half = core // 2, core % 2
        o = res[core]["out"]
        for j, qt in enumerate(_qtiles_for(half)):
            out[b_idx, qt * P : (qt + 1) * P, :] = o[j * P : (j + 1) * P, :]
    return out
